# revision 13
# baseline (speedup 1.0000x reference)
"""Camera2World Trainium2 Bass kernel.

out[b,n,i,h,w] = depth[b,n,h,w] * (p2p[b,n,i,0]*w + p2p[b,n,i,1]*h + p2p[b,n,i,2])
                 + p2p[b,n,i,3]          for i in 0..2

Data-parallel over the 24 (b,n) pairs: 3 pairs per core on 8 cores.

Per-core device kernel:
  - broadcast the 48 p2p coefficients to all 128 partitions with a K=1 matmul
    against a ones vector (PE)
  - rows[p] = c1*(h0+p) + c2 per (pair, i, block) via tensor_scalar (DVE, tiny)
  - per output tile [128, 960]:
      ACT : A = Identity(U * c0 + rows)          (scale/bias are [P,1] APs)
      DVE : m = A * depth
      POOL: o = m + c3
  - 12 depth loads + 36 output stores of 480 KB each per core
"""

from contextlib import ExitStack

import numpy as np

import concourse.bacc as bacc
import concourse.mybir as mybir
import concourse.tile as tile
from concourse.bass_utils import run_bass_kernel_spmd

F32 = mybir.dt.float32
B, N, H, W = 4, 6, 512, 960
NCORES = 8
PAIRS = B * N           # 24
PPC = PAIRS // NCORES   # 3 (b,n) pairs per core
PB = 128                # SBUF partitions
NB = H // PB            # 4 row blocks per image

_cached_nc = None


def _build_bass():
    nc = bacc.Bacc("TRN2", target_bir_lowering=False, debug=False, enable_asserts=False)
    depth = nc.dram_tensor("depth", [PPC * H, W], F32, kind="ExternalInput")
    p2p = nc.dram_tensor("p2p", [PB, PPC * 16], F32, kind="ExternalInput")
    out = nc.dram_tensor("out", [PPC * 3 * H, W], F32, kind="ExternalOutput")
    I32 = mybir.dt.int32

    mult = mybir.AluOpType.mult
    add = mybir.AluOpType.add
    ident = mybir.ActivationFunctionType.Identity

    with tile.TileContext(nc) as tc, ExitStack() as ctx:
        const = ctx.enter_context(tc.tile_pool(name="const", bufs=1))
        dpool = ctx.enter_context(tc.tile_pool(name="dp", bufs=3))
        mpool = ctx.enter_context(tc.tile_pool(name="mpl", bufs=6))
        jpool = ctx.enter_context(tc.tile_pool(name="jpl", bufs=2))
        opool = ctx.enter_context(tc.tile_pool(name="opl", bufs=4))

        # index grids generated on-chip: U[p, w] = w ; V[p, t] = p + 128t
        u_i32 = const.tile([PB, W], I32)
        nc.gpsimd.iota(u_i32[:], [[1, W]], base=0, channel_multiplier=0)
        u_sb = const.tile([PB, W], F32)
        nc.vector.tensor_copy(u_sb[:], u_i32[:])
        v_i32 = const.tile([PB, NB], I32)
        nc.gpsimd.iota(v_i32[:], [[PB, NB]], base=0, channel_multiplier=1)
        v_sb = const.tile([PB, NB], F32)
        nc.vector.tensor_copy(v_sb[:], v_i32[:])

        # coef[p, j] = p2p_flat[j] for every partition p (host-replicated)
        coef = const.tile([PB, PPC * 16], F32)
        nc.sync.dma_start(coef[:], p2p[:])

        # rows[p, (pair*3+i)*NB + t] = c1*(p + 128t) + c2
        rows = const.tile([PB, PPC * 3 * NB], F32)
        for pair in range(PPC):
            for i in range(3):
                g = (pair * 3 + i) * NB
                cb = 16 * pair + 4 * i
                nc.vector.tensor_scalar(
                    rows[:, g:g + NB], v_sb[:],
                    coef[:, cb + 1:cb + 2], coef[:, cb + 2:cb + 3],
                    mult, add)

        for pair in range(PPC):
            # whole-pair depth load: partition p, block t <- DRAM row t*128+p
            d = dpool.tile([PB, NB, W], F32)
            dview = depth[pair * H:(pair + 1) * H, :].rearrange(
                "(t p) w -> p t w", p=PB)
            nc.sync.dma_start(d[:], dview)
            oview = out[pair * 3 * H:(pair + 1) * 3 * H, :].rearrange(
                "(i h) w -> i h w", i=3)
            for t in range(NB):
                o = opool.tile([PB, 3, W], F32)
                for i in range(3):
                    cb = 16 * pair + 4 * i
                    g = (pair * 3 + i) * NB
                    m = mpool.tile([PB, W], F32)
                    junk = jpool.tile([PB, 1], F32)
                    nc.vector.affine_mul_reduce(
                        m[:], junk[:], u_sb[:], d[:, t, :],
                        scale=coef[:, cb:cb + 1],
                        bias=rows[:, g + t:g + t + 1])
                    nc.scalar.activation(
                        o[:, i, :], m[:], ident,
                        bias=coef[:, cb + 3:cb + 4],
                        scale=1.0)
                # combined store of the 3 channels of this row-block
                ov = oview[:, t * PB:(t + 1) * PB, :].rearrange("i p w -> p i w")
                nc.sync.dma_start(ov, o[:])
    nc.compile()
    return nc


def _make_in_maps(depth, p2p):
    dflat = np.ascontiguousarray(
        np.asarray(depth, dtype=np.float32)).reshape(PAIRS, H, W)
    pflat = np.ascontiguousarray(
        np.asarray(p2p, dtype=np.float32)).reshape(PAIRS, 16)
    in_maps = []
    for c in range(NCORES):
        sl = slice(c * PPC, (c + 1) * PPC)
        in_maps.append({
            "depth": np.ascontiguousarray(dflat[sl].reshape(PPC * H, W)),
            "p2p": np.ascontiguousarray(np.broadcast_to(
                pflat[sl].reshape(1, PPC * 16), (PB, PPC * 16))),
        })
    return in_maps


def _gather(results):
    outs = [np.asarray(r["out"]).reshape(PPC, 3, H, W) for r in results]
    return np.concatenate(outs, axis=0).reshape(B, N, 3, H, W)


def kernel(depth, p2p):
    global _cached_nc
    if _cached_nc is None:
        _cached_nc = _build_bass()
    in_maps = _make_in_maps(depth, p2p)
    res = run_bass_kernel_spmd(_cached_nc, in_maps, list(range(NCORES)))
    return _gather(res.results)


# revision 14
# speedup vs baseline: 1.1850x; 1.1850x over previous
"""Camera2World Trainium2 Bass kernel.

out[b,n,i,h,w] = depth[b,n,h,w] * (p2p[b,n,i,0]*w + p2p[b,n,i,1]*h + p2p[b,n,i,2])
                 + p2p[b,n,i,3]          for i in 0..2

Data-parallel over the 24 (b,n) pairs: 3 pairs per core on 8 cores.

Per-core device kernel:
  - broadcast the 48 p2p coefficients to all 128 partitions with a K=1 matmul
    against a ones vector (PE)
  - rows[p] = c1*(h0+p) + c2 per (pair, i, block) via tensor_scalar (DVE, tiny)
  - per output tile [128, 960]:
      ACT : A = Identity(U * c0 + rows)          (scale/bias are [P,1] APs)
      DVE : m = A * depth
      POOL: o = m + c3
  - 12 depth loads + 36 output stores of 480 KB each per core
"""

from contextlib import ExitStack

import numpy as np

import concourse.bacc as bacc
import concourse.mybir as mybir
import concourse.tile as tile
from concourse.bass_utils import run_bass_kernel_spmd

F32 = mybir.dt.float32
B, N, H, W = 4, 6, 512, 960
NCORES = 8
PAIRS = B * N           # 24
PPC = PAIRS // NCORES   # 3 (b,n) pairs per core
PB = 128                # SBUF partitions
NB = H // PB            # 4 row blocks per image

_cached_nc = None


def _build_bass():
    nc = bacc.Bacc("TRN2", target_bir_lowering=False, debug=False)
    depth = nc.dram_tensor("depth", [PPC * H, W], F32, kind="ExternalInput")
    p2p = nc.dram_tensor("p2p", [PB, PPC * 16], F32, kind="ExternalInput")
    out = nc.dram_tensor("out", [PPC * 3 * H, W], F32, kind="ExternalOutput")
    I32 = mybir.dt.int32

    mult = mybir.AluOpType.mult
    add = mybir.AluOpType.add
    ident = mybir.ActivationFunctionType.Identity

    with tile.TileContext(nc) as tc, ExitStack() as ctx:
        const = ctx.enter_context(tc.tile_pool(name="const", bufs=1))
        dpool = ctx.enter_context(tc.tile_pool(name="dp", bufs=3))
        mpool = ctx.enter_context(tc.tile_pool(name="mpl", bufs=6))
        jpool = ctx.enter_context(tc.tile_pool(name="jpl", bufs=2))
        opool = ctx.enter_context(tc.tile_pool(name="opl", bufs=4))

        # index grids generated on-chip: U[p, w] = w ; V[p, t] = p + 128t
        u_i32 = const.tile([PB, W], I32)
        nc.gpsimd.iota(u_i32[:], [[1, W]], base=0, channel_multiplier=0)
        u_sb = const.tile([PB, W], F32)
        nc.vector.tensor_copy(u_sb[:], u_i32[:])
        v_i32 = const.tile([PB, NB], I32)
        nc.gpsimd.iota(v_i32[:], [[PB, NB]], base=0, channel_multiplier=1)
        v_sb = const.tile([PB, NB], F32)
        nc.vector.tensor_copy(v_sb[:], v_i32[:])

        # coef[p, j] = p2p_flat[j] for every partition p (host-replicated)
        coef = const.tile([PB, PPC * 16], F32)
        nc.sync.dma_start(coef[:], p2p[:])

        # rows[p, (pair*3+i)*NB + t] = c1*(p + 128t) + c2
        rows = const.tile([PB, PPC * 3 * NB], F32)
        for pair in range(PPC):
            for i in range(3):
                g = (pair * 3 + i) * NB
                cb = 16 * pair + 4 * i
                nc.vector.tensor_scalar(
                    rows[:, g:g + NB], v_sb[:],
                    coef[:, cb + 1:cb + 2], coef[:, cb + 2:cb + 3],
                    mult, add)

        for pair in range(PPC):
            # whole-pair depth load: partition p, block t <- DRAM row t*128+p
            d = dpool.tile([PB, NB, W], F32)
            dview = depth[pair * H:(pair + 1) * H, :].rearrange(
                "(t p) w -> p t w", p=PB)
            nc.sync.dma_start(d[:], dview)
            oview = out[pair * 3 * H:(pair + 1) * 3 * H, :].rearrange(
                "(i h) w -> i h w", i=3)
            for t in range(NB):
                o = opool.tile([PB, 3, W], F32)
                for i in range(3):
                    cb = 16 * pair + 4 * i
                    g = (pair * 3 + i) * NB
                    m = mpool.tile([PB, W], F32)
                    junk = jpool.tile([PB, 1], F32)
                    nc.vector.affine_mul_reduce(
                        m[:], junk[:], u_sb[:], d[:, t, :],
                        scale=coef[:, cb:cb + 1],
                        bias=rows[:, g + t:g + t + 1])
                    nc.scalar.activation(
                        o[:, i, :], m[:], ident,
                        bias=coef[:, cb + 3:cb + 4],
                        scale=1.0)
                # combined store of the 3 channels of this row-block
                ov = oview[:, t * PB:(t + 1) * PB, :].rearrange("i p w -> p i w")
                nc.sync.dma_start(ov, o[:])
    nc.compile()
    return nc


def _make_in_maps(depth, p2p):
    dflat = np.ascontiguousarray(
        np.asarray(depth, dtype=np.float32)).reshape(PAIRS, H, W)
    pflat = np.ascontiguousarray(
        np.asarray(p2p, dtype=np.float32)).reshape(PAIRS, 16)
    in_maps = []
    for c in range(NCORES):
        sl = slice(c * PPC, (c + 1) * PPC)
        in_maps.append({
            "depth": np.ascontiguousarray(dflat[sl].reshape(PPC * H, W)),
            "p2p": np.ascontiguousarray(np.broadcast_to(
                pflat[sl].reshape(1, PPC * 16), (PB, PPC * 16))),
        })
    return in_maps


def _gather(results):
    outs = [np.asarray(r["out"]).reshape(PPC, 3, H, W) for r in results]
    return np.concatenate(outs, axis=0).reshape(B, N, 3, H, W)


def kernel(depth, p2p):
    global _cached_nc
    if _cached_nc is None:
        _cached_nc = _build_bass()
    in_maps = _make_in_maps(depth, p2p)
    res = run_bass_kernel_spmd(_cached_nc, in_maps, list(range(NCORES)))
    return _gather(res.results)
